# revision 77
# baseline (speedup 1.0000x reference)
"""Trainium2 Bass kernel for MoE head adapter (top-2 of 4 experts + proj).

Computes, for full inputs x[65536,256], w_gate[256,4], w1[4,256,512],
w2[4,512,256], w_proj[256,512], b_proj[512]:

    logits = x @ w_gate; top-2 softmax gates
    h = gelu(x @ w1[e]); y = sum_e g_e * (h_e @ w2[e]); out = y @ w_proj + b_proj

Sharding: data-parallel over tokens across 8 NeuronCores (8192 tokens/core,
weights replicated, no collectives).  Expert routing and fp8 were measured
and rejected (DMA-descriptor-rate-bound / error-gate-bound respectively:
fp8 e4m3 anywhere pushes rel-err to ~4e-2 vs the 2e-2 gate).

Single fused software-pipelined loop (vs a serial gating phase that left
the PE at the cold 1.2GHz HAM clock for ~93us).  Per iteration i the PE
stream is: gating-matmuls(i+1) -> up-proj(i) -> gate-transpose(i+1) ->
down-proj(i-1) -> out-proj(i-2), which keeps the PE dense and warm and
gives every cross-engine handoff a full-iteration (~17us) lead:
  - gating logits in exact fp32 on the PE (top-2 selection must be exact:
    bf16/fp22/hi-lo-split logits all statistically flip near-tied top-2
    picks, each flip costing ~0.1 max-rel error)
  - DVE top-2 softmax chain one super ahead; gelu batched [128,1024] on
    the ACT engine; gate-multiply batched on DVE
  - host supplies both f32 x (gating) and pre-cast bf16 x (experts);
    weights host-packed partition-major so each load is descriptor-cheap
    (DMA issue costs ~5-7ns/line ON the issuing engine)
  - ~10us of dense FD-512 warm-up matmuls on a constant bridge the
    startup DMA window at full PE duty, sandwiching the low-duty fp32
    gating block, so HAM never re-throttles to 1.2GHz
  - w1 on the gpsimd queue, w2 on sync after x0/x1, id/wg on scalar:
    keeps the scalar engine free for gelu(0) at startup
  - last super's yt/o casts on the ACT engine (the DVE FIFO backlog at
    the tail would stall the final out-proj by ~5us)
  - all expert/proj matmuls bf16 FD-512 at the ~215ns streaming roofline;
    steady state measures ~205ns/MM spacing, ~95% tensor-engine busy
Measured: ~315-319us (vs 344.5us checkpoint baseline); rel err 5.29e-3.
Rejected with evidence: fp8 e4m3 anywhere -> 4e-2+ vs the 2e-2 gate
(numpy sim matches device numerics exactly); routed top-2 dispatch is
DMA-descriptor-rate-bound and PE-one-hot dispatch exactly breaks even at
D=256; DVE 32x32 StreamTranspose scrambles this layout (semantics differ
from per-block in-place transpose at both bf16 and f32).
"""

import os
from contextlib import ExitStack

import numpy as np

import concourse.bass as bass
import concourse.tile as tile
from concourse import bacc, mybir
from concourse.bass_utils import run_bass_kernel_spmd

N, D, E, H, EMB = 65536, 256, 4, 512, 512
NCORES = 8
NSH = N // NCORES          # tokens per core
SUPER = 512                # tokens per super-tile
NSUP = NSH // SUPER
S_BLK = SUPER // 128       # 128-token sub-blocks per super-tile
KD = D // 128              # k-tiles over D
MH = H // 128              # m-tiles over H

F32 = mybir.dt.float32
AF = mybir.ActivationFunctionType
ALU = mybir.AluOpType
AX = mybir.AxisListType

# bf16 matmul operands: 1 cy/row on the PE + automatic fast-weight-load.
MM_DT = mybir.dt.bfloat16


def _moe_body(ctx: ExitStack, tc, xt, xb, wg, w1, w2, wp, ident, ones, out):
    nc = tc.nc

    const = ctx.enter_context(tc.tile_pool(name="const", bufs=1))
    sb = ctx.enter_context(tc.tile_pool(name="sb", bufs=2))
    ps_big = ctx.enter_context(tc.tile_pool(name="psbig", bufs=2, space="PSUM"))
    ps_yt = ctx.enter_context(tc.tile_pool(name="psyt", bufs=1, space="PSUM"))
    ps_sm = ctx.enter_context(tc.tile_pool(name="pssm", bufs=1, space="PSUM"))

    # --- replicated constants -------------------------------------------------
    # Spread across DMA queues so startup isn't serialized on one queue:
    # scalar: warm-up operand + id + wg (needed first), gpsimd: w1 (e-major,
    # first expert first) + wp, scalar: w2 (needed one super later).
    # warm-up operand arrives by DMA (~8.6us): a DVE memset would make it
    # available ~1.5us earlier, but then the warm-up drains before the
    # DMA-anchored pipeline-fill stall at ~30us and the HAM re-throttles —
    # measured net worse
    ones_sb = const.tile([128, SUPER], MM_DT)
    nc.scalar.dma_start(ones_sb[:], ones[:])
    id_sb = const.tile([128, 128], F32)
    nc.scalar.dma_start(id_sb[:], ident[:])
    wg_sb = const.tile([128, KD, E], F32)
    nc.scalar.dma_start(wg_sb[:], wg.rearrange("(k p) e -> p k e", p=128))
    # w1/w2/wp arrive host-packed in the exact SBUF layout: one 128-line DMA
    # each (8KB contiguous per partition).  DMA descriptor generation costs
    # ~5-7ns per line on the issuing engine, so the naive per-tile loads
    # (~3000 x 512B lines) would burn ~20us of engine time at startup.
    w1_sb = const.tile([128, E, KD, H], MM_DT)
    w2_sb = const.tile([128, E, MH, D], MM_DT)
    wp_sb = const.tile([128, KD, EMB], MM_DT)
    for e in range(E):
        for k in range(KD):
            nc.gpsimd.dma_start(
                w1_sb[:, e, k, :],
                w1[:, (e * KD + k) * H : (e * KD + k + 1) * H],
            )
    nc.gpsimd.dma_start(
        wp_sb[:], wp.rearrange("p (k m) -> p k m", k=KD)
    )

    # gate broadcast runs on the (otherwise idle) gpsimd engine
    from concourse import library_config

    nc.gpsimd.load_library(library_config.mlp)

    # ---------------- pipeline stage helpers ---------------------------------
    def dma_loads(T):
        """Stream super T's x: f32 (gating) + bf16 (experts)."""
        tok0 = T * SUPER
        xt32 = sb.tile([128, KD, SUPER], F32, tag="xt32", bufs=3, name=f"xt32_{T}")
        nc.sync.dma_start(
            xt32[:],
            xt[:, tok0 : tok0 + SUPER].rearrange("(k p) t -> p k t", p=128),
        )
        xbt = sb.tile([128, KD, SUPER], MM_DT, tag="xb", bufs=3, name=f"xb_{T}")
        nc.sync.dma_start(
            xbt[:],
            xb[:, tok0 : tok0 + SUPER].rearrange("(k p) t -> p k t", p=128),
        )
        return xt32, xbt

    def gating_mm_thunks(T, xt32):
        """Exact-f32 gating logit matmuls for super T, as deferred thunks so
        they can interleave between big bf16 matmuls (hides their fp32
        LDWEIGHTS, which can't fast-weight-load, behind streaming MMs)."""
        lg_ps = ps_sm.tile([128, S_BLK, E], F32, tag="lgps", bufs=1, name=f"lg{T}")

        def mk(s):
            def go():
                for k in range(KD):
                    nc.tensor.matmul(
                        lg_ps[:, s, :],
                        xt32[:, k, s * 128 : (s + 1) * 128],
                        wg_sb[:, k, :],
                        start=(k == 0),
                        stop=(k == KD - 1),
                    )
            return go

        return lg_ps, [mk(s) for s in range(S_BLK)]

    def gating_chain(T, lg_ps):
        """DVE top-2 softmax on super T's logits -> dense gates [tok, s, e]."""
        def bc(t):
            return t[:].broadcast_to([128, S_BLK, E])

        lg = sb.tile([128, S_BLK, E], F32, tag="lg")
        nc.vector.tensor_copy(lg[:], lg_ps[:])
        m1 = sb.tile([128, S_BLK, 1], F32, tag="m1")
        nc.vector.reduce_max(m1[:], lg[:], axis=AX.X)
        t0 = sb.tile([128, S_BLK, E], F32, tag="t0")
        nc.vector.tensor_tensor(t0[:], lg[:], bc(m1), op=ALU.is_equal)
        t1 = sb.tile([128, S_BLK, E], F32, tag="t1")
        nc.vector.tensor_scalar_mul(t1[:], t0[:], -1e9)
        t2 = sb.tile([128, S_BLK, E], F32, tag="t2")
        nc.vector.tensor_tensor(t2[:], lg[:], t1[:], op=ALU.add)
        m2 = sb.tile([128, S_BLK, 1], F32, tag="m2")
        nc.vector.reduce_max(m2[:], t2[:], axis=AX.X)
        t3 = sb.tile([128, S_BLK, E], F32, tag="t3")
        nc.vector.tensor_tensor(t3[:], lg[:], bc(m2), op=ALU.is_ge)
        t4 = sb.tile([128, S_BLK, E], F32, tag="t4")
        nc.vector.tensor_tensor(t4[:], lg[:], bc(m1), op=ALU.subtract)
        t5 = sb.tile([128, S_BLK, E], F32, tag="t5")
        nc.scalar.activation(t5[:], t4[:], AF.Exp)
        t6 = sb.tile([128, S_BLK, E], F32, tag="t6")
        nc.vector.tensor_tensor(t6[:], t5[:], t3[:], op=ALU.mult)
        den = sb.tile([128, S_BLK, 1], F32, tag="den")
        nc.vector.reduce_sum(den[:], t6[:], axis=AX.X)
        rcp = sb.tile([128, S_BLK, 1], F32, tag="rcp")
        nc.vector.reciprocal(rcp[:], den[:])
        g_sb = sb.tile([128, S_BLK, E], F32, tag="g")
        nc.vector.tensor_tensor(g_sb[:], t6[:], bc(rcp), op=ALU.mult)
        return g_sb

    def gating_bcast(T, g_sb):
        """DVE 32x32 block-transpose + per-expert gather + gpsimd broadcast
        -> G[e] [128,512].  (Keeps the transpose off the tensor engine.)

        After the block transpose of g[p, (s,e)], expert e's gates for
        s-block s sit at partition 32a + 4s + e, free j, for token
        s*128 + 32a + j."""
        gt_ps = ps_sm.tile([S_BLK * E, 128], F32, tag="gtps", bufs=1, name=f"gt{T}")
        nc.tensor.transpose(gt_ps[:], g_sb[:, :, :], id_sb[:])
        gt_sb = sb.tile([S_BLK * E, 128], MM_DT, tag="gtsb", bufs=2)
        nc.vector.tensor_copy(gt_sb[:], gt_ps[:])
        Gs = []
        for e in range(E):
            gte = sb.tile([1, SUPER], MM_DT, tag="gte", bufs=8)
            nc.sync.dma_start(gte[:], gt_sb[e::E, :])
            G = sb.tile([128, SUPER], MM_DT, tag="Gsb", bufs=8, name=f"G{T}_{e}")
            nc.gpsimd.partition_broadcast(G[:], gte[:], channels=128)
            Gs.append(G)
        return Gs

    def up_proj(T, xbt, Gs, gate_thunks=()):
        """Up-proj + gelu + gate-mult for super T -> hgg[e] [128, MH, 512] bf16."""
        gate_thunks = list(gate_thunks)
        hggs = []
        for e in range(E):
            hgg = sb.tile(
                [128, MH, SUPER], MM_DT, tag="hgg", bufs=8, name=f"hgg{T}_{e}"
            )
            hggs.append(hgg)
            for mp in range(MH // 2):
                h_ps = ps_big.tile([128, 2, SUPER], F32, tag="big", bufs=2)
                for mi in range(2):
                    m = mp * 2 + mi
                    for k in range(KD):
                        nc.tensor.matmul(
                            h_ps[:, mi, :],
                            w1_sb[:, e, k, m * 128 : (m + 1) * 128],
                            xbt[:, k, :],
                            start=(k == 0),
                            stop=(k == KD - 1),
                        )
                hg = sb.tile([128, 2, SUPER], MM_DT, tag="hg", bufs=4)
                nc.scalar.activation(hg[:], h_ps[:], AF.Gelu)
                nc.vector.tensor_tensor(
                    hgg[:, mp * 2 : mp * 2 + 2, :],
                    hg[:],
                    Gs[e][:, None, :].broadcast_to([128, 2, SUPER]),
                    op=ALU.mult,
                )
        return hggs

    def down_proj(T, hggs):
        """Down-proj for super T: yt[kd, tok] bf16."""
        # note: down-proj MMs pace ~225ns vs up-proj's ~200ns; measured to be
        # independent of md loop position — intrinsic to the 16-deep PSUM
        # accumulation (94% of these MMs read-modify-write PSUM), not fixable
        # by reordering
        yt_ps = ps_yt.tile([128, KD, SUPER], F32, tag="yt", bufs=1, name=f"yt{T}")
        for e in range(E):
            for m in range(MH):
                for md in range(KD):
                    nc.tensor.matmul(
                        yt_ps[:, md, :],
                        w2_sb[:, e, m, md * 128 : (md + 1) * 128],
                        hggs[e][:, m, :],
                        start=(e == 0 and m == 0),
                        stop=(e == E - 1 and m == MH - 1),
                    )
        yt_sb = sb.tile([128, KD, SUPER], MM_DT, tag="ytsb", bufs=3, name=f"yts{T}")
        if T == NSUP - 1:
            # last super: cast on the ACT engine — the DVE FIFO backlog at the
            # kernel tail would delay this by ~5us and stall the final out-proj
            nc.scalar.copy(yt_sb[:], yt_ps[:])
        else:
            nc.vector.tensor_copy(yt_sb[:], yt_ps[:])
        return yt_sb

    def out_proj(T, yt_sb):
        """Output projection for super T, DMA to HBM (bf16)."""
        tok0 = T * SUPER
        for sp in range(S_BLK // 2):
            o_ps = ps_big.tile([128, 2, EMB], F32, tag="big", bufs=2)
            for si in range(2):
                s = sp * 2 + si
                for kd in range(KD):
                    nc.tensor.matmul(
                        o_ps[:, si, :],
                        yt_sb[:, kd, s * 128 : (s + 1) * 128],
                        wp_sb[:, kd, :],
                        start=(kd == 0),
                        stop=(kd == KD - 1),
                    )
            o_sb = sb.tile([128, 2, EMB], MM_DT, tag="osb", bufs=4)
            if T == NSUP - 1 and sp == S_BLK // 2 - 1:
                nc.scalar.copy(o_sb[:], o_ps[:])
            else:
                nc.vector.tensor_copy(o_sb[:], o_ps[:])
            q = nc.scalar if sp % 2 == 0 else nc.sync
            for si in range(2):
                s = sp * 2 + si
                q.dma_start(
                    out[tok0 + s * 128 : tok0 + (s + 1) * 128, :], o_sb[:, si, :]
                )

    # ---------------- fused pipeline -----------------------------------------
    # prologue: supers 0+1 x prefetch, super 0 gating.
    # ~10us of dense FD-512 bf16 matmuls on a tiny constant (single stationary,
    # no LDWEIGHTS churn) while the startup DMAs stream: pushes the PE through
    # a HAM SHORT window at full duty so real compute starts at 2.4GHz instead
    # of the cold 1.2GHz default, and bridges the DMA wait so it never
    # re-throttles.
    warm_ps = ps_yt.tile([128, KD, SUPER], F32, tag="yt", bufs=1, name="warmjunk")
    for j in range(24):
        nc.tensor.matmul(
            warm_ps[:, 0, :], ones_sb[:, :128], ones_sb[:], start=True, stop=True
        )
    xt32_0, xb_0 = dma_loads(0)
    xt32_1, xb_1 = dma_loads(1)
    # w2 on the sync queue after super 0/1's x (first needed one super later;
    # keeping it off the scalar queue leaves that engine free for gelu(0))
    for e in range(E):
        for m in range(MH):
            nc.sync.dma_start(
                w2_sb[:, e, m, :],
                w2[:, (e * MH + m) * D : (e * MH + m + 1) * D],
            )
    lg0, thunks0 = gating_mm_thunks(0, xt32_0)
    for t in thunks0:
        t()
    # a few more dense warm MMs right after the low-duty fp32 gating block:
    # keeps PE duty high through the window where up-proj(0) still waits on
    # w1/xb DMAs, so the HAM doesn't re-throttle to 1.2GHz
    for j in range(8):
        nc.tensor.matmul(
            warm_ps[:, 0, :], ones_sb[:, :128], ones_sb[:], start=True, stop=True
        )
    g0 = gating_chain(0, lg0)
    G_cur = gating_bcast(0, g0)

    xb_cur, xt32_nxt, xb_nxt = xb_0, xt32_1, xb_1
    hgg_prev = None          # hgg of super i-1
    yt_q = []                # yt_sb tiles awaiting out-proj (supers i-2, i-1)

    for i in range(NSUP):
        if i + 2 < NSUP:
            nxt2 = dma_loads(i + 2)
        if i + 1 < NSUP:
            lg_nxt, thunks_nxt = gating_mm_thunks(i + 1, xt32_nxt)
        else:
            thunks_nxt = []
        # gating MMs for i+1 as one contiguous block before the bf16 stream
        # (interleaving fp32 MMs into the bf16 stream destabilizes HAM/FWL)
        for t in thunks_nxt:
            t()
        hgg_cur = up_proj(i, xb_cur, G_cur)
        # gating DVE chain for i+1
        if i + 1 < NSUP:
            g_nxt = gating_chain(i + 1, lg_nxt)
        # down-proj super i-1
        if hgg_prev is not None:
            yt_q.append(down_proj(i - 1, hgg_prev))
        if i == 0:
            # iteration 0 has no down-proj to cover the iter-0 gelu/DVE
            # pipeline-fill latency (~4us): keep the PE busy with junk MMs
            # instead of idling into a HAM re-throttle
            for j in range(22):
                nc.tensor.matmul(
                    warm_ps[:, 0, :], ones_sb[:, :128], ones_sb[:],
                    start=True, stop=True,
                )
        # transpose/broadcast for i+1 after down-proj: the DVE chain (which
        # trails the gate-mults in the DVE FIFO) then has a full down-proj
        # of lead before the PE transpose needs it
        if i + 1 < NSUP:
            G_nxt = gating_bcast(i + 1, g_nxt)
        # out-proj super i-2
        if len(yt_q) == 2:
            out_proj(i - 2, yt_q.pop(0))
        hgg_prev = hgg_cur
        if i + 1 < NSUP:
            xb_cur = xb_nxt
            G_cur = G_nxt
        if i + 2 < NSUP:
            xt32_nxt, xb_nxt = nxt2

    # epilogue
    yt_q.append(down_proj(NSUP - 1, hgg_prev))
    out_proj(NSUP - 2, yt_q.pop(0))
    out_proj(NSUP - 1, yt_q.pop(0))


_PROGRAM = None


def _build():
    global _PROGRAM
    if _PROGRAM is not None:
        return _PROGRAM
    nc = bacc.Bacc("TRN2", target_bir_lowering=False, debug=False, num_devices=NCORES)
    ones = nc.dram_tensor("ones", [128, SUPER], MM_DT, kind="ExternalInput").ap()
    xt = nc.dram_tensor("xt", [D, NSH], F32, kind="ExternalInput").ap()
    xb = nc.dram_tensor("xb", [D, NSH], MM_DT, kind="ExternalInput").ap()
    wg = nc.dram_tensor("w_gate", [D, E], F32, kind="ExternalInput").ap()
    w1 = nc.dram_tensor("w1", [128, KD * E * H], MM_DT, kind="ExternalInput").ap()
    w2 = nc.dram_tensor("w2", [128, MH * E * D], MM_DT, kind="ExternalInput").ap()
    wp = nc.dram_tensor("w_proj", [128, KD * EMB], MM_DT, kind="ExternalInput").ap()
    ident = nc.dram_tensor("ident", [128, 128], F32, kind="ExternalInput").ap()
    out = nc.dram_tensor("out", [NSH, EMB], MM_DT, kind="ExternalOutput").ap()
    with tile.TileContext(nc) as tc, ExitStack() as ctx:
        _moe_body(ctx, tc, xt, xb, wg, w1, w2, wp, ident, ones, out)
    nc.compile()
    _PROGRAM = nc
    return nc


def _install_trace_shim():
    """Recreate the antenv.axon_hooks NTFF profile hook (missing in this image)."""
    import sys
    import types
    import contextlib
    import ctypes

    if "antenv.axon_hooks" in sys.modules:
        return
    so_path = "/opt/axon/libaxon_pjrt.so"
    lib = ctypes.CDLL(so_path)
    lib.axon_start_nrt_profile.argtypes = [ctypes.POINTER(ctypes.c_int64), ctypes.c_size_t]
    lib.axon_start_nrt_profile.restype = ctypes.c_int64
    lib.axon_stop_nrt_profile.argtypes = [ctypes.c_char_p]
    lib.axon_stop_nrt_profile.restype = ctypes.c_int64

    @contextlib.contextmanager
    def _hook(output_dir, device_ids):
        import jax

        jax.devices()
        if device_ids:
            ids = (ctypes.c_int64 * len(device_ids))(*device_ids)
            rc = lib.axon_start_nrt_profile(ids, len(device_ids))
        else:
            rc = lib.axon_start_nrt_profile(None, 0)
        if rc != 0:
            raise RuntimeError(f"axon_start_nrt_profile rc={rc}")
        try:
            yield
        finally:
            n = lib.axon_stop_nrt_profile(str(output_dir).encode())
            if n <= 0:
                print(f"profile: {n} ntff files written to {output_dir}")

    mod = types.ModuleType("antenv.axon_hooks")
    _state = {"hook": _hook}
    mod.get_axon_ntff_profile_hook = lambda: _state["hook"]
    mod.set_axon_ntff_profile_hook = lambda h: _state.__setitem__("hook", h)
    sys.modules["antenv.axon_hooks"] = mod

    import concourse.bass_utils as bu

    bu.upload_artifacts = lambda tmpdir: f"local:{tmpdir}"


def kernel(x, w_gate, w1, w2, w_proj, b_proj):
    nc = _build()
    import ml_dtypes

    bf16 = ml_dtypes.bfloat16
    ident = np.eye(128, dtype=np.float32)
    # pack weights into the exact SBUF layouts (partition-major, 8KB/partition)
    w1_b = np.ascontiguousarray(
        w1.astype(bf16).reshape(E, KD, 128, H).transpose(2, 0, 1, 3).reshape(128, -1)
    )
    w2_b = np.ascontiguousarray(
        w2.astype(bf16).reshape(E, MH, 128, D).transpose(2, 0, 1, 3).reshape(128, -1)
    )
    wp_b = np.ascontiguousarray(
        w_proj.astype(bf16).reshape(KD, 128, EMB).transpose(1, 0, 2).reshape(128, -1)
    )
    in_maps = []
    for i in range(NCORES):
        xs = x[i * NSH : (i + 1) * NSH].T
        in_maps.append(
            {
                "ones": np.ones((128, SUPER), dtype=bf16),
                "xt": np.ascontiguousarray(xs),
                "xb": np.ascontiguousarray(xs.astype(bf16)),
                "w_gate": np.ascontiguousarray(w_gate),
                "w1": w1_b,
                "w2": w2_b,
                "w_proj": wp_b,
                "ident": ident,
            }
        )
    trace = bool(int(os.environ.get("MOE_TRACE", "0")))
    if trace:
        _install_trace_shim()
        import tempfile

        tmpdir = os.environ.get("MOE_TRACE_DIR") or tempfile.mkdtemp(prefix="moe_trace_")
        res = run_bass_kernel_spmd(
            nc, in_maps, list(range(NCORES)), trace=True, tmpdir=tmpdir,
            trace_cores=[0],
        )
        print(f"HW exec time: {res.exec_time_ns} ns")
        print(f"trace dir: {tmpdir}")
        kernel.last_result = res
    else:
        res = run_bass_kernel_spmd(nc, in_maps, list(range(NCORES)))
    full = np.concatenate(
        [res.results[i]["out"].astype(np.float32) for i in range(NCORES)], axis=0
    )
    return full + b_proj[None, :]


# revision 79
# speedup vs baseline: 1.0008x; 1.0008x over previous
"""Trainium2 Bass kernel for MoE head adapter (top-2 of 4 experts + proj).

Computes, for full inputs x[65536,256], w_gate[256,4], w1[4,256,512],
w2[4,512,256], w_proj[256,512], b_proj[512]:

    logits = x @ w_gate; top-2 softmax gates
    h = gelu(x @ w1[e]); y = sum_e g_e * (h_e @ w2[e]); out = y @ w_proj + b_proj

Sharding: data-parallel over tokens across 8 NeuronCores (8192 tokens/core,
weights replicated, no collectives).  Expert routing and fp8 were measured
and rejected (DMA-descriptor-rate-bound / error-gate-bound respectively:
fp8 e4m3 anywhere pushes rel-err to ~4e-2 vs the 2e-2 gate).

Single fused software-pipelined loop (vs a serial gating phase that left
the PE at the cold 1.2GHz HAM clock for ~93us).  Per iteration i the PE
stream is: gating-matmuls(i+1) -> up-proj(i) -> gate-transpose(i+1) ->
down-proj(i-1) -> out-proj(i-2), which keeps the PE dense and warm and
gives every cross-engine handoff a full-iteration (~17us) lead:
  - gating logits in exact fp32 on the PE (top-2 selection must be exact:
    bf16/fp22/hi-lo-split logits all statistically flip near-tied top-2
    picks, each flip costing ~0.1 max-rel error)
  - DVE top-2 softmax chain one super ahead; gelu batched [128,1024] on
    the ACT engine; gate-multiply batched on DVE
  - host supplies both f32 x (gating) and pre-cast bf16 x (experts);
    weights host-packed partition-major so each load is descriptor-cheap
    (DMA issue costs ~5-7ns/line ON the issuing engine)
  - ~10us of dense FD-512 warm-up matmuls on a constant bridge the
    startup DMA window at full PE duty, sandwiching the low-duty fp32
    gating block, so HAM never re-throttles to 1.2GHz
  - w1 on the gpsimd queue, w2 on sync after x0/x1, id/wg on scalar:
    keeps the scalar engine free for gelu(0) at startup
  - last super's yt/o casts on the ACT engine (the DVE FIFO backlog at
    the tail would stall the final out-proj by ~5us)
  - all expert/proj matmuls bf16 FD-512 at the ~215ns streaming roofline;
    steady state measures ~205ns/MM spacing, ~95% tensor-engine busy
Measured: ~315-319us (vs 344.5us checkpoint baseline); rel err 5.29e-3.
Rejected with evidence: fp8 e4m3 anywhere -> 4e-2+ vs the 2e-2 gate
(numpy sim matches device numerics exactly); routed top-2 dispatch is
DMA-descriptor-rate-bound and PE-one-hot dispatch exactly breaks even at
D=256; DVE 32x32 StreamTranspose scrambles this layout (semantics differ
from per-block in-place transpose at both bf16 and f32).
"""

import os
from contextlib import ExitStack

import numpy as np

import concourse.bass as bass
import concourse.tile as tile
from concourse import bacc, mybir
from concourse.bass_utils import run_bass_kernel_spmd

N, D, E, H, EMB = 65536, 256, 4, 512, 512
NCORES = 8
NSH = N // NCORES          # tokens per core
SUPER = 512                # tokens per super-tile
NSUP = NSH // SUPER
S_BLK = SUPER // 128       # 128-token sub-blocks per super-tile
KD = D // 128              # k-tiles over D
MH = H // 128              # m-tiles over H

F32 = mybir.dt.float32
AF = mybir.ActivationFunctionType
ALU = mybir.AluOpType
AX = mybir.AxisListType

# bf16 matmul operands: 1 cy/row on the PE + automatic fast-weight-load.
MM_DT = mybir.dt.bfloat16


def _moe_body(ctx: ExitStack, tc, xt, xb, wg, w1, w2, wp, ident, ones, out):
    nc = tc.nc

    const = ctx.enter_context(tc.tile_pool(name="const", bufs=1))
    sb = ctx.enter_context(tc.tile_pool(name="sb", bufs=2))
    ps_big = ctx.enter_context(tc.tile_pool(name="psbig", bufs=2, space="PSUM"))
    ps_yt = ctx.enter_context(tc.tile_pool(name="psyt", bufs=1, space="PSUM"))
    ps_sm = ctx.enter_context(tc.tile_pool(name="pssm", bufs=1, space="PSUM"))

    # --- replicated constants -------------------------------------------------
    # Spread across DMA queues so startup isn't serialized on one queue:
    # scalar: warm-up operand + id + wg (needed first), gpsimd: w1 (e-major,
    # first expert first) + wp, scalar: w2 (needed one super later).
    # warm-up operand arrives by DMA (~8.6us): a DVE memset would make it
    # available ~1.5us earlier, but then the warm-up drains before the
    # DMA-anchored pipeline-fill stall at ~30us and the HAM re-throttles —
    # measured net worse
    ones_sb = const.tile([128, SUPER], MM_DT)
    nc.scalar.dma_start(ones_sb[:], ones[:])
    id_sb = const.tile([128, 128], F32)
    nc.scalar.dma_start(id_sb[:], ident[:])
    wg_sb = const.tile([128, KD, E], F32)
    nc.scalar.dma_start(wg_sb[:], wg.rearrange("(k p) e -> p k e", p=128))
    # w1/w2/wp arrive host-packed in the exact SBUF layout: one 128-line DMA
    # each (8KB contiguous per partition).  DMA descriptor generation costs
    # ~5-7ns per line on the issuing engine, so the naive per-tile loads
    # (~3000 x 512B lines) would burn ~20us of engine time at startup.
    w1_sb = const.tile([128, E, KD, H], MM_DT)
    w2_sb = const.tile([128, E, MH, D], MM_DT)
    wp_sb = const.tile([128, KD, EMB], MM_DT)
    for e in range(E):
        for k in range(KD):
            nc.gpsimd.dma_start(
                w1_sb[:, e, k, :],
                w1[:, (e * KD + k) * H : (e * KD + k + 1) * H],
            )
    nc.gpsimd.dma_start(
        wp_sb[:], wp.rearrange("p (k m) -> p k m", k=KD)
    )

    # gate broadcast runs on the (otherwise idle) gpsimd engine
    from concourse import library_config

    nc.gpsimd.load_library(library_config.mlp)

    # ---------------- pipeline stage helpers ---------------------------------
    def dma_loads(T):
        """Stream super T's x: f32 (gating) + bf16 (experts)."""
        tok0 = T * SUPER
        xt32 = sb.tile([128, KD, SUPER], F32, tag="xt32", bufs=3, name=f"xt32_{T}")
        nc.sync.dma_start(
            xt32[:],
            xt[:, tok0 : tok0 + SUPER].rearrange("(k p) t -> p k t", p=128),
        )
        xbt = sb.tile([128, KD, SUPER], MM_DT, tag="xb", bufs=3, name=f"xb_{T}")
        nc.sync.dma_start(
            xbt[:],
            xb[:, tok0 : tok0 + SUPER].rearrange("(k p) t -> p k t", p=128),
        )
        return xt32, xbt

    def gating_mm_thunks(T, xt32):
        """Exact-f32 gating logit matmuls for super T, as deferred thunks so
        they can interleave between big bf16 matmuls (hides their fp32
        LDWEIGHTS, which can't fast-weight-load, behind streaming MMs)."""
        lg_ps = ps_sm.tile([128, S_BLK, E], F32, tag="lgps", bufs=1, name=f"lg{T}")

        def mk(s):
            def go():
                for k in range(KD):
                    nc.tensor.matmul(
                        lg_ps[:, s, :],
                        xt32[:, k, s * 128 : (s + 1) * 128],
                        wg_sb[:, k, :],
                        start=(k == 0),
                        stop=(k == KD - 1),
                    )
            return go

        return lg_ps, [mk(s) for s in range(S_BLK)]

    def gating_chain(T, lg_ps):
        """DVE top-2 softmax on super T's logits -> dense gates [tok, s, e]."""
        def bc(t):
            return t[:].broadcast_to([128, S_BLK, E])

        lg = sb.tile([128, S_BLK, E], F32, tag="lg")
        nc.vector.tensor_copy(lg[:], lg_ps[:])
        m1 = sb.tile([128, S_BLK, 1], F32, tag="m1")
        nc.vector.reduce_max(m1[:], lg[:], axis=AX.X)
        t0 = sb.tile([128, S_BLK, E], F32, tag="t0")
        nc.vector.tensor_tensor(t0[:], lg[:], bc(m1), op=ALU.is_equal)
        t1 = sb.tile([128, S_BLK, E], F32, tag="t1")
        nc.vector.tensor_scalar_mul(t1[:], t0[:], -1e9)
        t2 = sb.tile([128, S_BLK, E], F32, tag="t2")
        nc.vector.tensor_tensor(t2[:], lg[:], t1[:], op=ALU.add)
        m2 = sb.tile([128, S_BLK, 1], F32, tag="m2")
        nc.vector.reduce_max(m2[:], t2[:], axis=AX.X)
        t3 = sb.tile([128, S_BLK, E], F32, tag="t3")
        nc.vector.tensor_tensor(t3[:], lg[:], bc(m2), op=ALU.is_ge)
        t4 = sb.tile([128, S_BLK, E], F32, tag="t4")
        nc.vector.tensor_tensor(t4[:], lg[:], bc(m1), op=ALU.subtract)
        t5 = sb.tile([128, S_BLK, E], F32, tag="t5")
        nc.scalar.activation(t5[:], t4[:], AF.Exp)
        t6 = sb.tile([128, S_BLK, E], F32, tag="t6")
        nc.vector.tensor_tensor(t6[:], t5[:], t3[:], op=ALU.mult)
        den = sb.tile([128, S_BLK, 1], F32, tag="den")
        nc.vector.reduce_sum(den[:], t6[:], axis=AX.X)
        rcp = sb.tile([128, S_BLK, 1], F32, tag="rcp")
        nc.vector.reciprocal(rcp[:], den[:])
        g_sb = sb.tile([128, S_BLK, E], F32, tag="g")
        nc.vector.tensor_tensor(g_sb[:], t6[:], bc(rcp), op=ALU.mult)
        return g_sb

    def gating_bcast(T, g_sb):
        """DVE 32x32 block-transpose + per-expert gather + gpsimd broadcast
        -> G[e] [128,512].  (Keeps the transpose off the tensor engine.)

        After the block transpose of g[p, (s,e)], expert e's gates for
        s-block s sit at partition 32a + 4s + e, free j, for token
        s*128 + 32a + j."""
        gt_ps = ps_sm.tile([S_BLK * E, 128], F32, tag="gtps", bufs=1, name=f"gt{T}")
        nc.tensor.transpose(gt_ps[:], g_sb[:, :, :], id_sb[:])
        gt_sb = sb.tile([S_BLK * E, 128], MM_DT, tag="gtsb", bufs=2)
        nc.vector.tensor_copy(gt_sb[:], gt_ps[:])
        Gs = []
        for e in range(E):
            gte = sb.tile([1, SUPER], MM_DT, tag="gte", bufs=8)
            nc.sync.dma_start(gte[:], gt_sb[e::E, :])
            G = sb.tile([128, SUPER], MM_DT, tag="Gsb", bufs=8, name=f"G{T}_{e}")
            nc.gpsimd.partition_broadcast(G[:], gte[:], channels=128)
            Gs.append(G)
        return Gs

    def up_proj(T, xbt, Gs, gate_thunks=()):
        """Up-proj + gelu + gate-mult for super T -> hgg[e] [128, MH, 512] bf16."""
        gate_thunks = list(gate_thunks)
        hggs = []
        for e in range(E):
            hgg = sb.tile(
                [128, MH, SUPER], MM_DT, tag="hgg", bufs=8, name=f"hgg{T}_{e}"
            )
            hggs.append(hgg)
            for mp in range(MH // 2):
                h_ps = ps_big.tile([128, 2, SUPER], F32, tag="big", bufs=2)
                for mi in range(2):
                    m = mp * 2 + mi
                    for k in range(KD):
                        nc.tensor.matmul(
                            h_ps[:, mi, :],
                            w1_sb[:, e, k, m * 128 : (m + 1) * 128],
                            xbt[:, k, :],
                            start=(k == 0),
                            stop=(k == KD - 1),
                        )
                hg = sb.tile([128, 2, SUPER], MM_DT, tag="hg", bufs=4)
                nc.scalar.activation(hg[:], h_ps[:], AF.Gelu)
                nc.vector.tensor_tensor(
                    hgg[:, mp * 2 : mp * 2 + 2, :],
                    hg[:],
                    Gs[e][:, None, :].broadcast_to([128, 2, SUPER]),
                    op=ALU.mult,
                )
        return hggs

    def down_proj(T, hggs):
        """Down-proj for super T: yt[kd, tok] bf16."""
        # note: down-proj MMs pace ~225ns vs up-proj's ~200ns; measured to be
        # independent of md loop position — intrinsic to the 16-deep PSUM
        # accumulation (94% of these MMs read-modify-write PSUM), not fixable
        # by reordering
        yt_ps = ps_yt.tile([128, KD, SUPER], F32, tag="yt", bufs=1, name=f"yt{T}")
        for e in range(E):
            for m in range(MH):
                for md in range(KD):
                    nc.tensor.matmul(
                        yt_ps[:, md, :],
                        w2_sb[:, e, m, md * 128 : (md + 1) * 128],
                        hggs[e][:, m, :],
                        start=(e == 0 and m == 0),
                        stop=(e == E - 1 and m == MH - 1),
                    )
        yt_sb = sb.tile([128, KD, SUPER], MM_DT, tag="ytsb", bufs=3, name=f"yts{T}")
        if T == NSUP - 1:
            # last super: cast on the ACT engine — the DVE FIFO backlog at the
            # kernel tail would delay this by ~5us and stall the final out-proj
            nc.scalar.copy(yt_sb[:], yt_ps[:])
        else:
            nc.vector.tensor_copy(yt_sb[:], yt_ps[:])
        return yt_sb

    def out_proj(T, yt_sb):
        """Output projection for super T, DMA to HBM (bf16)."""
        tok0 = T * SUPER
        for sp in range(S_BLK // 2):
            o_ps = ps_big.tile([128, 2, EMB], F32, tag="big", bufs=2)
            for si in range(2):
                s = sp * 2 + si
                for kd in range(KD):
                    nc.tensor.matmul(
                        o_ps[:, si, :],
                        yt_sb[:, kd, s * 128 : (s + 1) * 128],
                        wp_sb[:, kd, :],
                        start=(kd == 0),
                        stop=(kd == KD - 1),
                    )
            o_sb = sb.tile([128, 2, EMB], MM_DT, tag="osb", bufs=4)
            if T == NSUP - 1 and sp == S_BLK // 2 - 1:
                nc.scalar.copy(o_sb[:], o_ps[:])
            else:
                nc.vector.tensor_copy(o_sb[:], o_ps[:])
            q = nc.scalar if sp % 2 == 0 else nc.sync
            for si in range(2):
                s = sp * 2 + si
                q.dma_start(
                    out[tok0 + s * 128 : tok0 + (s + 1) * 128, :], o_sb[:, si, :]
                )

    # ---------------- fused pipeline -----------------------------------------
    # prologue: supers 0+1 x prefetch, super 0 gating.
    # ~10us of dense FD-512 bf16 matmuls on a tiny constant (single stationary,
    # no LDWEIGHTS churn) while the startup DMAs stream: pushes the PE through
    # a HAM SHORT window at full duty so real compute starts at 2.4GHz instead
    # of the cold 1.2GHz default, and bridges the DMA wait so it never
    # re-throttles.
    warm_ps = ps_yt.tile([128, KD, SUPER], F32, tag="yt", bufs=1, name="warmjunk")
    for j in range(24):
        nc.tensor.matmul(
            warm_ps[:, 0, :], ones_sb[:, :128], ones_sb[:], start=True, stop=True
        )
    xt32_0, xb_0 = dma_loads(0)
    xt32_1, xb_1 = dma_loads(1)
    # w2 on the sync queue after super 0/1's x (first needed one super later;
    # keeping it off the scalar queue leaves that engine free for gelu(0))
    for e in range(E):
        for m in range(MH):
            nc.sync.dma_start(
                w2_sb[:, e, m, :],
                w2[:, (e * MH + m) * D : (e * MH + m + 1) * D],
            )
    lg0, thunks0 = gating_mm_thunks(0, xt32_0)
    for t in thunks0:
        t()
    # a few more dense warm MMs right after the low-duty fp32 gating block:
    # keeps PE duty high through the window where up-proj(0) still waits on
    # w1/xb DMAs, so the HAM doesn't re-throttle to 1.2GHz
    for j in range(8):
        nc.tensor.matmul(
            warm_ps[:, 0, :], ones_sb[:, :128], ones_sb[:], start=True, stop=True
        )
    g0 = gating_chain(0, lg0)
    G_cur = gating_bcast(0, g0)

    xb_cur, xt32_nxt, xb_nxt = xb_0, xt32_1, xb_1
    hgg_prev = None          # hgg of super i-1
    yt_q = []                # yt_sb tiles awaiting out-proj (supers i-2, i-1)

    for i in range(NSUP):
        if i + 2 < NSUP:
            nxt2 = dma_loads(i + 2)
        if i + 1 < NSUP:
            lg_nxt, thunks_nxt = gating_mm_thunks(i + 1, xt32_nxt)
        else:
            thunks_nxt = []
        # gating MMs for i+1 as one contiguous block before the bf16 stream
        # (interleaving fp32 MMs into the bf16 stream destabilizes HAM/FWL)
        for t in thunks_nxt:
            t()
        hgg_cur = up_proj(i, xb_cur, G_cur)
        # gating DVE chain for i+1
        if i + 1 < NSUP:
            g_nxt = gating_chain(i + 1, lg_nxt)
        # down-proj super i-1
        if hgg_prev is not None:
            yt_q.append(down_proj(i - 1, hgg_prev))
        if i == 0:
            # iteration 0 has no down-proj to cover the iter-0 gelu/DVE
            # pipeline-fill latency (~4us): keep the PE busy with junk MMs
            # instead of idling into a HAM re-throttle
            for j in range(22):
                nc.tensor.matmul(
                    warm_ps[:, 0, :], ones_sb[:, :128], ones_sb[:],
                    start=True, stop=True,
                )
        # transpose/broadcast for i+1 after down-proj: the DVE chain (which
        # trails the gate-mults in the DVE FIFO) then has a full down-proj
        # of lead before the PE transpose needs it
        if i + 1 < NSUP:
            G_nxt = gating_bcast(i + 1, g_nxt)
        # out-proj super i-2
        if len(yt_q) == 2:
            out_proj(i - 2, yt_q.pop(0))
        hgg_prev = hgg_cur
        if i + 1 < NSUP:
            xb_cur = xb_nxt
            G_cur = G_nxt
        if i + 2 < NSUP:
            xt32_nxt, xb_nxt = nxt2

    # epilogue
    yt_q.append(down_proj(NSUP - 1, hgg_prev))
    out_proj(NSUP - 2, yt_q.pop(0))
    out_proj(NSUP - 1, yt_q.pop(0))


_PROGRAM = None


def _build():
    global _PROGRAM
    if _PROGRAM is not None:
        return _PROGRAM
    nc = bacc.Bacc("TRN2", target_bir_lowering=False, debug=False, num_devices=NCORES)
    ones = nc.dram_tensor("ones", [128, SUPER], MM_DT, kind="ExternalInput").ap()
    xt = nc.dram_tensor("xt", [D, NSH], F32, kind="ExternalInput").ap()
    xb = nc.dram_tensor("xb", [D, NSH], MM_DT, kind="ExternalInput").ap()
    wg = nc.dram_tensor("w_gate", [D, E], F32, kind="ExternalInput").ap()
    w1 = nc.dram_tensor("w1", [128, KD * E * H], MM_DT, kind="ExternalInput").ap()
    w2 = nc.dram_tensor("w2", [128, MH * E * D], MM_DT, kind="ExternalInput").ap()
    wp = nc.dram_tensor("w_proj", [128, KD * EMB], MM_DT, kind="ExternalInput").ap()
    ident = nc.dram_tensor("ident", [128, 128], F32, kind="ExternalInput").ap()
    out = nc.dram_tensor("out", [NSH, EMB], MM_DT, kind="ExternalOutput").ap()
    with tile.TileContext(nc) as tc, ExitStack() as ctx:
        _moe_body(ctx, tc, xt, xb, wg, w1, w2, wp, ident, ones, out)
    nc.compile()
    _PROGRAM = nc
    return nc


def _install_trace_shim():
    """Recreate the antenv.axon_hooks NTFF profile hook (missing in this image)."""
    import sys
    import types
    import contextlib
    import ctypes

    if "antenv.axon_hooks" in sys.modules:
        return
    so_path = "/opt/axon/libaxon_pjrt.so"
    lib = ctypes.CDLL(so_path)
    lib.axon_start_nrt_profile.argtypes = [ctypes.POINTER(ctypes.c_int64), ctypes.c_size_t]
    lib.axon_start_nrt_profile.restype = ctypes.c_int64
    lib.axon_stop_nrt_profile.argtypes = [ctypes.c_char_p]
    lib.axon_stop_nrt_profile.restype = ctypes.c_int64

    @contextlib.contextmanager
    def _hook(output_dir, device_ids):
        import jax

        jax.devices()
        if device_ids:
            ids = (ctypes.c_int64 * len(device_ids))(*device_ids)
            rc = lib.axon_start_nrt_profile(ids, len(device_ids))
        else:
            rc = lib.axon_start_nrt_profile(None, 0)
        if rc != 0:
            raise RuntimeError(f"axon_start_nrt_profile rc={rc}")
        try:
            yield
        finally:
            n = lib.axon_stop_nrt_profile(str(output_dir).encode())
            if n <= 0:
                print(f"profile: {n} ntff files written to {output_dir}")

    mod = types.ModuleType("antenv.axon_hooks")
    _state = {"hook": _hook}
    mod.get_axon_ntff_profile_hook = lambda: _state["hook"]
    mod.set_axon_ntff_profile_hook = lambda h: _state.__setitem__("hook", h)
    sys.modules["antenv.axon_hooks"] = mod

    import concourse.bass_utils as bu

    bu.upload_artifacts = lambda tmpdir: f"local:{tmpdir}"


def kernel(x, w_gate, w1, w2, w_proj, b_proj):
    nc = _build()
    import ml_dtypes

    bf16 = ml_dtypes.bfloat16
    ident = np.eye(128, dtype=np.float32)
    # pack weights into the exact SBUF layouts (partition-major, 8KB/partition)
    w1_b = np.ascontiguousarray(
        w1.astype(bf16).reshape(E, KD, 128, H).transpose(2, 0, 1, 3).reshape(128, -1)
    )
    w2_b = np.ascontiguousarray(
        w2.astype(bf16).reshape(E, MH, 128, D).transpose(2, 0, 1, 3).reshape(128, -1)
    )
    wp_b = np.ascontiguousarray(
        w_proj.astype(bf16).reshape(KD, 128, EMB).transpose(1, 0, 2).reshape(128, -1)
    )
    in_maps = []
    for i in range(NCORES):
        xs = x[i * NSH : (i + 1) * NSH].T
        in_maps.append(
            {
                "ones": np.ones((128, SUPER), dtype=bf16),
                "xt": np.ascontiguousarray(xs),
                "xb": np.ascontiguousarray(xs.astype(bf16)),
                "w_gate": np.ascontiguousarray(w_gate),
                "w1": w1_b,
                "w2": w2_b,
                "w_proj": wp_b,
                "ident": ident,
            }
        )
    trace = bool(int(os.environ.get("MOE_TRACE", "0")))
    if trace:
        _install_trace_shim()
        import tempfile

        tmpdir = os.environ.get("MOE_TRACE_DIR") or tempfile.mkdtemp(prefix="moe_trace_")
        res = run_bass_kernel_spmd(
            nc, in_maps, list(range(NCORES)), trace=True, tmpdir=tmpdir,
            trace_cores=[0],
        )
        print(f"HW exec time: {res.exec_time_ns} ns")
        print(f"trace dir: {tmpdir}")
        kernel.last_result = res
    else:
        res = run_bass_kernel_spmd(nc, in_maps, list(range(NCORES)))
    full = np.concatenate(
        [res.results[i]["out"].astype(np.float32) for i in range(NCORES)], axis=0
    )
    return full + b_proj[None, :]
